# revision 1
# baseline (speedup 1.0000x reference)
"""IrrepsLinear Trainium2 kernel: y = per-irrep-block x @ W / sqrt(mul).

Irreps layout: 256x0e + 128x1o + 64x2e -> blocks of width 256*1, 128*3, 64*5.
Data-parallel over 8 NeuronCores: each core gets 12500 nodes.

Strategy:
  - fp16 DRAM IO halves HBM traffic (the roofline-binding resource);
    matmuls run fp16 x fp16 -> fp32 PSUM, evac casts back to fp16.
  - Host pre-permutes features so each 128-row K-group of the matmuls is
    one contiguous DRAM block per node-window: every DMA is monolithic
    and contiguous (strided/chunked DMA patterns measured far slower,
    and pushing DMA harder trips the HW DMA-utilization throttle).
  - Window schedule [2900 x4, 900]: big windows keep DMA at peak rate;
    the small final window shrinks the end-of-run store drain, which has
    nothing left to overlap it.
  - Block2's five 64-wide m-components: pairs (m0,m1), (m2,m3) via a
    128x128 block-diagonal W2 stationary (full PE width); m4 plain.
  - 8 one-bank PSUM tiles rotate per 512-slice; evac alternates DVE/ACT
    engines; loads on the SP HWDGE ring, stores on the ACT ring.
"""

import numpy as np

NCORES = 8
N_TOTAL = 100000
NSH = N_TOTAL // NCORES   # 12500 nodes per core
D = 960
MMW = 512                 # matmul slice width (= one fp32 PSUM bank)

WINDOWS = [2900, 2900, 2900, 2900, 900]
assert sum(WINDOWS) == NSH
OFFS = np.concatenate([[0], np.cumsum(WINDOWS)[:-1]]).tolist()

DT_IO = "float16"
_BUILD_CACHE = {}


def _perm():
    p = list(range(256))
    for m in range(3):
        p += [256 + 3 * i + m for i in range(128)]
    for m in range(5):
        p += [640 + 5 * i + m for i in range(64)]
    return np.asarray(p, dtype=np.int64)

_PERM = _perm()


def _build_program():
    import concourse.bass as bass  # noqa: F401
    import concourse.bacc as bacc
    import concourse.mybir as mybir
    import concourse.tile as tile

    key = (DT_IO, MMW, tuple(WINDOWS), "v10")
    if key in _BUILD_CACHE:
        return _BUILD_CACHE[key]

    dt = getattr(mybir.dt, DT_IO)
    f32 = mybir.dt.float32

    nc = bacc.Bacc(
        "TRN2", target_bir_lowering=False, debug=False, enable_asserts=False
    )
    xa = nc.dram_tensor("xa", [128, 7 * NSH], dt, kind="ExternalInput").ap()
    xbd = nc.dram_tensor("xb", [64, NSH], dt, kind="ExternalInput").ap()
    w0 = nc.dram_tensor("w0", [256, 256], dt, kind="ExternalInput").ap()
    w1 = nc.dram_tensor("w1", [128, 128], dt, kind="ExternalInput").ap()
    w2d = nc.dram_tensor("w2d", [128, 128], dt, kind="ExternalInput").ap()
    w2s = nc.dram_tensor("w2s", [64, 64], dt, kind="ExternalInput").ap()
    ya = nc.dram_tensor("ya", [128, 7 * NSH], dt, kind="ExternalOutput").ap()
    ybd = nc.dram_tensor("yb", [64, NSH], dt, kind="ExternalOutput").ap()

    with tile.TileContext(nc) as tc:
        with (
            tc.tile_pool(name="const", bufs=1) as cpool,
            tc.tile_pool(name="xin", bufs=2) as xpool,
            tc.tile_pool(name="yst", bufs=2) as ypool,
            tc.tile_pool(name="ps", bufs=8, space="PSUM") as pspool,
        ):
            w0t0 = cpool.tile([128, 256], dt, name="w0t0", tag="w0t0")
            nc.sync.dma_start(w0t0[:], w0[0:128, :])
            w0t1 = cpool.tile([128, 256], dt, name="w0t1", tag="w0t1")
            nc.sync.dma_start(w0t1[:], w0[128:256, :])
            w1t = cpool.tile([128, 128], dt, name="w1t", tag="w1t")
            nc.sync.dma_start(w1t[:], w1[:, :])
            w2dt = cpool.tile([128, 128], dt, name="w2dt", tag="w2dt")
            nc.sync.dma_start(w2dt[:], w2d[:, :])
            w2st = cpool.tile([64, 64], dt, name="w2st", tag="w2st")
            nc.sync.dma_start(w2st[:], w2s[:, :])

            n_evac = 0

            def evac(dst, src):
                nonlocal n_evac
                n_evac += 1
                if n_evac % 2:
                    nc.vector.tensor_copy(dst, src)
                else:
                    nc.scalar.copy(dst, src)

            for wi, (c0, sw) in enumerate(zip(OFFS, WINDOWS)):
                xat = xpool.tile([128, 7 * sw], dt, name=f"xa{wi}", tag="xa")
                nc.sync.dma_start(xat[:], xa[:, 7 * c0 : 7 * (c0 + sw)])
                xbt = xpool.tile([64, sw], dt, name=f"xb{wi}", tag="xb")
                nc.sync.dma_start(xbt[:], xbd[:, c0 : c0 + sw])
                yat = ypool.tile([128, 7 * sw], dt, name=f"ya{wi}", tag="ya")
                ybt = ypool.tile([64, sw], dt, name=f"yb{wi}", tag="yb")

                slices = [
                    (i * MMW, min((i + 1) * MMW, sw))
                    for i in range((sw + MMW - 1) // MMW)
                ]
                for lo, hi in slices:
                    n = hi - lo

                    def pst(nm):
                        return pspool.tile(
                            [128, MMW], f32, name=f"{nm}_{wi}_{lo}", tag="ps"
                        )

                    for ob in range(2):
                        ps = pst(f"ps_b0_{ob}")
                        oc = slice(128 * ob, 128 * (ob + 1))
                        nc.tensor.matmul(
                            ps[:, :n], w0t0[:, oc], xat[:, 0 * sw + lo : 0 * sw + hi],
                            start=True, stop=False,
                        )
                        nc.tensor.matmul(
                            ps[:, :n], w0t1[:, oc], xat[:, 1 * sw + lo : 1 * sw + hi],
                            start=False, stop=True,
                        )
                        evac(yat[:, ob * sw + lo : ob * sw + hi], ps[:, :n])

                    for m in range(3):
                        ps = pst(f"ps_b1_{m}")
                        t = 2 + m
                        nc.tensor.matmul(
                            ps[:, :n], w1t[:], xat[:, t * sw + lo : t * sw + hi],
                            start=True, stop=True,
                        )
                        evac(yat[:, t * sw + lo : t * sw + hi], ps[:, :n])

                    for g in range(2):
                        ps = pst(f"ps_b2_{g}")
                        t = 5 + g
                        nc.tensor.matmul(
                            ps[:, :n], w2dt[:], xat[:, t * sw + lo : t * sw + hi],
                            start=True, stop=True,
                        )
                        evac(yat[:, t * sw + lo : t * sw + hi], ps[:, :n])

                    ps = pst("ps_b2_4")
                    nc.tensor.matmul(
                        ps[0:64, :n], w2st[:], xbt[:, lo:hi],
                        start=True, stop=True,
                    )
                    evac(ybt[:, lo:hi], ps[0:64, :n])

                nc.scalar.dma_start(ya[:, 7 * c0 : 7 * (c0 + sw)], yat[:])
                nc.scalar.dma_start(ybd[:, c0 : c0 + sw], ybt[:])

    nc.compile()
    _BUILD_CACHE[key] = nc
    return nc


TRACE = False
LAST_RESULT = None


def kernel(x, W0, W1, W2):
    from concourse import bass_utils

    nc = _build_program()

    npdt = np.float16 if DT_IO == "float16" else None
    if npdt is None:
        import ml_dtypes
        npdt = ml_dtypes.bfloat16

    w0s = (np.asarray(W0, np.float32) / np.sqrt(256.0)).astype(npdt)
    w1s = (np.asarray(W1, np.float32) / np.sqrt(128.0)).astype(npdt)
    w2 = (np.asarray(W2, np.float32) / np.sqrt(64.0)).astype(npdt)
    w2d = np.zeros((128, 128), dtype=npdt)
    w2d[0:64, 0:64] = w2
    w2d[64:128, 64:128] = w2

    xh = np.asarray(x)[:, _PERM].astype(npdt)
    A = xh.reshape(NCORES, NSH, D)
    blocks = []
    for c0, sw in zip(OFFS, WINDOWS):
        blk = A[:, c0 : c0 + sw, :896].reshape(NCORES, sw, 7, 128)
        blocks.append(blk.transpose(0, 3, 2, 1).reshape(NCORES, 128, 7 * sw))
    xa_all = np.ascontiguousarray(np.concatenate(blocks, axis=2))
    xb_all = np.ascontiguousarray(A[:, :, 896:].transpose(0, 2, 1))

    in_maps = []
    for c in range(NCORES):
        in_maps.append({
            "xa": xa_all[c], "xb": xb_all[c],
            "w0": w0s, "w1": w1s, "w2d": w2d, "w2s": w2,
        })

    res = bass_utils.run_bass_kernel_spmd(
        nc, in_maps, core_ids=list(range(NCORES)), trace=TRACE
    )
    global LAST_RESULT
    LAST_RESULT = res

    out = np.empty((N_TOTAL, D), dtype=np.float32)
    Yp = np.empty((NCORES, NSH, D), dtype=np.float32)
    for c in range(NCORES):
        yac = res.results[c]["ya"]    # [128, 7*NSH]
        ybc = res.results[c]["yb"]    # [64, NSH]
        for c0, sw in zip(OFFS, WINDOWS):
            blk = yac[:, 7 * c0 : 7 * (c0 + sw)].reshape(128, 7, sw)
            Yp[c, c0 : c0 + sw, :896] = blk.transpose(2, 1, 0).reshape(sw, 896)
        Yp[c, :, 896:] = ybc.T
    out[:, _PERM] = Yp.reshape(N_TOTAL, D)
    return out

